# revision 14
# baseline (speedup 1.0000x reference)
"""MemoryNet kernel for 8 Trainium2 NeuronCores.

Math (per batch b):
    qn = q / ||q||_L2-over-L          (column-wise norm over sequence axis)
    kn = k / ||k||_L2-over-L
    qk[d, e] = sum_l qn[l, d] * kn[l, e]          # [D, D] channel cross-cov
    sm = softmax(qk, axis=e)
    out[l, d] = sum_e v[l, e] * sm[d, e]          # v @ sm^T

Key identity: qk = (q^T k) * rnq[d] * rnk[e] with rnq = 1/||q[:,d]||,
rnk = 1/||k[:,e]|| — normalization never touches the big [L, D] tensors.
sq_q comes from diag(q^T q), sq_k from diag(k^T k), both computed on the
PE alongside q^T k.

Sharding (8 cores, B=4): core c -> batch b = c//2, L-half h = c%2.
Each core receives full q_b, k_b (needed for the full-L contraction) and
its half of v_b; computes its half of out_b.  No collectives.

Layout trick: HBM rows are only 512B, so a [l-on-partitions] tile load
would use 512B DMA descriptors (4x off line rate).  Instead each SBUF
partition p holds CONSECUTIVE HBM rows (16 for q/k, 8 for v/out), giving
4-8KB contiguous descriptors.  The L-contraction is order-free, so
matmul L-"tiles" are the interleaved row sets {16p + t}; accumulating
over t=0..15 still sums over all L exactly.

Precision: q/k are cast to fp16 on the host — they only feed the
softmax logits, where |logits| <= 1; fp16's 11-bit mantissa keeps the
logit error ~1e-5, far below fp32 output tolerance, and halves q/k HBM
traffic.  The v-path (v transposes + v @ sm^T) stays full fp32 (PE fp32
= exact 2-pass mode).

Since |qk| <= 1, softmax runs without max-subtraction.  The reference's
max(norm, 1e-12) clamp is a no-op at these magnitudes (norms ~sqrt(2048)).
"""

import numpy as np

import concourse.bass as bass
import concourse.bacc as bacc
import concourse.mybir as mybir
import concourse.tile as tile
from concourse.bass_utils import run_bass_kernel_spmd
from concourse.masks import make_identity

F32 = mybir.dt.float32
F16 = mybir.dt.float16
B, L, D = 4, 2048, 128
P = 128                    # SBUF partitions
NCORES = 8
LV = L // 2                # v/out rows per core
NT = L // P                # 16 q/k L-groups per core
NVT = LV // P              # 8 v L-groups per core
TPC = 4                    # L-groups per DMA chunk (q/k)
NCHUNK = NT // TPC         # 4 q/k chunks


def _build() -> bass.Bass:
    nc = bacc.Bacc("TRN2", target_bir_lowering=False, debug=False)
    q_d = nc.dram_tensor("q", [L, D], F16, kind="ExternalInput")
    k_d = nc.dram_tensor("k", [L, D], F16, kind="ExternalInput")
    v_d = nc.dram_tensor("v", [LV, D], F32, kind="ExternalInput")
    o_d = nc.dram_tensor("out", [LV, D], F32, kind="ExternalOutput")

    # flat views: partition p <- consecutive HBM rows (big DMA descriptors)
    q_r = q_d.rearrange("(p t) d -> p t d", p=P)   # [128, 16, 128], row 16p+t
    k_r = k_d.rearrange("(p t) d -> p t d", p=P)
    v_r = v_d.rearrange("(p s) d -> p s d", p=P)   # [128, 8, 128], row 8p+s
    o_r = o_d.rearrange("(p s) d -> p s d", p=P)

    with tile.TileContext(nc) as tc:
        with (
            tc.tile_pool(name="persist", bufs=1) as persist,
            tc.tile_pool(name="work", bufs=2) as work,
            tc.tile_pool(name="ps_acc", bufs=1, space="PSUM") as ps_acc,
            tc.tile_pool(name="ps_mid", bufs=1, space="PSUM") as ps_mid,
            tc.tile_pool(name="ps_mm", bufs=2, space="PSUM") as ps_mm,
        ):
            ident = persist.tile([P, P], F32)
            make_identity(nc, ident)
            ones_row = persist.tile([1, P], F32)
            nc.vector.memset(ones_row, 1.0)

            # ---- loads (both HWDGE rings in parallel) ----
            # q on the SP ring, k on the ACT ring; 4KB/partition descriptors
            sb_q = persist.tile([P, NT, D], F16)
            sb_k = persist.tile([P, NT, D], F16)
            nc.scalar.dma_start(out=sb_k, in_=k_r[:, :, :])
            nc.sync.dma_start(out=sb_q, in_=q_r[:, :, :])
            # v: fp32 exact
            sb_v = persist.tile([P, NVT, D], F32)
            nc.sync.dma_start(out=sb_v, in_=v_r[:, :, :])

            # HAM warm-up: ~3.4us of dummy PE work during the DMA wait so
            # the real matmuls run at 2.4GHz instead of the cold 1.2GHz
            wsrc = persist.tile([P, 2 * D], F16)
            nc.vector.memset(wsrc, 0.0)
            for w in range(16):
                ps_w = ps_mid.tile([P, 2 * D], F32, tag="mid", name=f"ps_w{w}")
                nc.tensor.matmul(ps_w, lhsT=wsrc[:, 0:D], rhs=wsrc,
                                 start=True, stop=True)

            # ---- phase 1 (PE): k^T k first, then q^T k / q^T q ----
            # one PSUM bank per accumulation group: a start=True clear is
            # bank-granular and wipes a sibling group's has_written bits.
            # kk finishes first so the whole rnk sub-chain (diag, sqrt,
            # reciprocal on DVE/ACT) overlaps the remaining matmuls.
            ps_qk = ps_acc.tile([P, D], F32)
            ps_qq = ps_acc.tile([P, D], F32)
            ps_kk = ps_acc.tile([P, D], F32)
            for t in range(NT):
                kt = sb_k[:, t, :]
                nc.tensor.matmul(ps_kk, lhsT=kt, rhs=kt,
                                 start=(t == 0), stop=(t == NT - 1))
            for t in range(NT):
                qt = sb_q[:, t, :]
                kt = sb_k[:, t, :]
                st, sp = (t == 0), (t == NT - 1)
                nc.tensor.matmul(ps_qk, lhsT=qt, rhs=kt, start=st, stop=sp)
                nc.tensor.matmul(ps_qq, lhsT=qt, rhs=qt, start=st, stop=sp)

            # warm the ACT Sqrt/Exp tables off the critical path (cold-table
            # ACTIVATE costs ~1-2us)
            # (input tied to sb_k so the scheduler cannot hoist these ACT
            # ops ahead of the k DMA issue on the same engine)
            warm2 = work.tile([P, 1], F32, name="warm2")
            nc.scalar.activation(out=warm2, in_=sb_k[:, 0, 0:1],
                                 func=mybir.ActivationFunctionType.Sqrt)
            warm3 = work.tile([P, 1], F32, name="warm3")
            nc.scalar.activation(out=warm3, in_=sb_k[:, 0, 0:1],
                                 func=mybir.ActivationFunctionType.Exp)

            # rnk sub-chain (DVE/ACT; overlaps the qk/qq matmuls above)
            dk = work.tile([P, P], F32)
            nc.vector.tensor_mul(dk, ps_kk, ident)
            sq_k = work.tile([P, 1], F32)
            nc.vector.reduce_sum(sq_k, dk, axis=mybir.AxisListType.X)
            nk = work.tile([P, 1], F32)
            nc.scalar.activation(out=nk, in_=sq_k,
                                 func=mybir.ActivationFunctionType.Sqrt)
            rnk = work.tile([P, 1], F32)
            nc.vector.reciprocal(rnk, nk)

            # rnq sub-chain
            dq = work.tile([P, P], F32)
            nc.vector.tensor_mul(dq, ps_qq, ident)
            sq_q = work.tile([P, 1], F32)
            nc.vector.reduce_sum(sq_q, dq, axis=mybir.AxisListType.X)
            nq = work.tile([P, 1], F32)
            nc.scalar.activation(out=nq, in_=sq_q,
                                 func=mybir.ActivationFunctionType.Sqrt)
            rnq = work.tile([P, 1], F32)
            nc.vector.reciprocal(rnq, nq)

            # rnk as a broadcast matrix: transpose to a row, outer with ones
            ps_rT = ps_mid.tile([1, P], F32, tag="mid", name="ps_rT")
            nc.tensor.transpose(ps_rT, rnk, ident)
            rnk_row = work.tile([1, P], F32)
            nc.vector.tensor_copy(rnk_row, ps_rT)
            ps_bc = ps_mid.tile([P, P], F32, tag="mid", name="ps_bc")
            nc.tensor.matmul(ps_bc, lhsT=ones_row, rhs=rnk_row,
                             start=True, stop=True)
            rnk_b = work.tile([P, P], F32)
            nc.vector.tensor_copy(rnk_b, ps_bc)

            # ---- v transposes (PE, fp32 exact): vT[:, s, :] = v_s^T ----
            sb_vT = persist.tile([P, NVT, D], F32)
            for s in range(NVT):
                ps_vT = ps_mm.tile([P, P], F32, tag="vT")
                nc.tensor.transpose(ps_vT, sb_v[:, s, :], ident)
                nc.vector.tensor_copy(sb_vT[:, s, :], ps_vT)

            # ---- softmax over e (free axis) ----
            qks = work.tile([P, P], F32)
            nc.vector.tensor_mul(qks, ps_qk, rnk_b)
            E = work.tile([P, P], F32)
            S = work.tile([P, 1], F32)
            nc.scalar.activation(out=E, in_=qks,
                                 func=mybir.ActivationFunctionType.Exp,
                                 scale=rnq, accum_out=S)
            rS = work.tile([P, 1], F32)
            nc.vector.reciprocal(rS, S)
            sm = work.tile([P, P], F32)
            nc.vector.tensor_scalar_mul(sm, E, rS)
            ps_smT = ps_mid.tile([P, P], F32, tag="mid", name="ps_smT")
            nc.tensor.transpose(ps_smT, sm, ident)
            smT = persist.tile([P, P], F32)   # [e, d]
            nc.vector.tensor_copy(smT, ps_smT)

            # ---- phase 2 (PE, fp32): out_s[l, d] = vT_s^T @ smT ----
            sb_out = persist.tile([P, NVT, D], F32)
            for s in range(NVT):
                ps_o = ps_mm.tile([P, P], F32, tag="po")
                nc.tensor.matmul(ps_o, lhsT=sb_vT[:, s, :], rhs=smT,
                                 start=True, stop=True)
                nc.vector.tensor_copy(sb_out[:, s, :], ps_o)
                if s == NVT // 2 - 1:
                    nc.scalar.dma_start(out=o_r[:, 0:NVT // 2, :],
                                        in_=sb_out[:, 0:NVT // 2, :])
                elif s == NVT - 1:
                    nc.scalar.dma_start(out=o_r[:, NVT // 2:, :],
                                        in_=sb_out[:, NVT // 2:, :])
    nc.compile()
    return nc


_CACHE: dict = {}


def _get_nc() -> bass.Bass:
    if "nc" not in _CACHE:
        _CACHE["nc"] = _build()
    return _CACHE["nc"]


def kernel(q: np.ndarray, k: np.ndarray, v: np.ndarray) -> np.ndarray:
    nc = _get_nc()
    q = np.ascontiguousarray(np.asarray(q, dtype=np.float32).astype(np.float16))
    k = np.ascontiguousarray(np.asarray(k, dtype=np.float32).astype(np.float16))
    v = np.ascontiguousarray(np.asarray(v, dtype=np.float32))
    in_maps = []
    for c in range(NCORES):
        b, h = divmod(c, 2)
        in_maps.append({
            "q": q[b],
            "k": k[b],
            "v": np.ascontiguousarray(v[b, h * LV:(h + 1) * LV]),
        })
    res = run_bass_kernel_spmd(nc, in_maps, list(range(NCORES))).results
    out = np.empty((B, L, D), dtype=np.float32)
    for c in range(NCORES):
        b, h = divmod(c, 2)
        out[b, h * LV:(h + 1) * LV] = res[c]["out"]
    return out


# revision 15
# speedup vs baseline: 1.0339x; 1.0339x over previous
"""MemoryNet kernel for 8 Trainium2 NeuronCores.

Math (per batch b):
    qn = q / ||q||_L2-over-L          (column-wise norm over sequence axis)
    kn = k / ||k||_L2-over-L
    qk[d, e] = sum_l qn[l, d] * kn[l, e]          # [D, D] channel cross-cov
    sm = softmax(qk, axis=e)
    out[l, d] = sum_e v[l, e] * sm[d, e]          # v @ sm^T

Key identity: qk = (q^T k) * rnq[d] * rnk[e] with rnq = 1/||q[:,d]||,
rnk = 1/||k[:,e]|| — normalization never touches the big [L, D] tensors.
sq_q comes from diag(q^T q), sq_k from diag(k^T k), both computed on the
PE alongside q^T k.

Sharding (8 cores, B=4): core c -> batch b = c//2, L-half h = c%2.
Each core receives full q_b, k_b (needed for the full-L contraction) and
its half of v_b; computes its half of out_b.  No collectives.

Layout trick: HBM rows are only 512B, so a [l-on-partitions] tile load
would use 512B DMA descriptors (4x off line rate).  Instead each SBUF
partition p holds CONSECUTIVE HBM rows (16 for q/k, 8 for v/out), giving
4-8KB contiguous descriptors.  The L-contraction is order-free, so
matmul L-"tiles" are the interleaved row sets {16p + t}; accumulating
over t=0..15 still sums over all L exactly.

Precision: q/k are cast to fp16 on the host — they only feed the
softmax logits, where |logits| <= 1; fp16's 11-bit mantissa keeps the
logit error ~1e-5, far below fp32 output tolerance, and halves q/k HBM
traffic.  The v-path (v transposes + v @ sm^T) stays full fp32 (PE fp32
= exact 2-pass mode).

Since |qk| <= 1, softmax runs without max-subtraction.  The reference's
max(norm, 1e-12) clamp is a no-op at these magnitudes (norms ~sqrt(2048)).
"""

import numpy as np

import concourse.bass as bass
import concourse.bacc as bacc
import concourse.mybir as mybir
import concourse.tile as tile
from concourse.bass_utils import run_bass_kernel_spmd
from concourse.masks import make_identity

F32 = mybir.dt.float32
F16 = mybir.dt.float16
B, L, D = 4, 2048, 128
P = 128                    # SBUF partitions
NCORES = 8
LV = L // 2                # v/out rows per core
NT = L // P                # 16 q/k L-groups per core
NVT = LV // P              # 8 v L-groups per core
TPC = 4                    # L-groups per DMA chunk (q/k)
NCHUNK = NT // TPC         # 4 q/k chunks


def _build() -> bass.Bass:
    nc = bacc.Bacc("TRN2", target_bir_lowering=False, debug=False)
    q_d = nc.dram_tensor("q", [L, D], F16, kind="ExternalInput")
    k_d = nc.dram_tensor("k", [L, D], F16, kind="ExternalInput")
    v_d = nc.dram_tensor("v", [LV, D], F32, kind="ExternalInput")
    o_d = nc.dram_tensor("out", [LV, D], F32, kind="ExternalOutput")

    # flat views: partition p <- consecutive HBM rows (big DMA descriptors)
    q_r = q_d.rearrange("(p t) d -> p t d", p=P)   # [128, 16, 128], row 16p+t
    k_r = k_d.rearrange("(p t) d -> p t d", p=P)
    v_r = v_d.rearrange("(p s) d -> p s d", p=P)   # [128, 8, 128], row 8p+s
    o_r = o_d.rearrange("(p s) d -> p s d", p=P)

    with tile.TileContext(nc) as tc:
        with (
            tc.tile_pool(name="persist", bufs=1) as persist,
            tc.tile_pool(name="work", bufs=2) as work,
            tc.tile_pool(name="ps_acc", bufs=1, space="PSUM") as ps_acc,
            tc.tile_pool(name="ps_mid", bufs=1, space="PSUM") as ps_mid,
            tc.tile_pool(name="ps_mm", bufs=2, space="PSUM") as ps_mm,
        ):
            ident = persist.tile([P, P], F32)
            make_identity(nc, ident)
            ones_row = persist.tile([1, P], F32)
            nc.vector.memset(ones_row, 1.0)

            # ---- loads (both HWDGE rings in parallel) ----
            # q on the SP ring, k on the ACT ring; 4KB/partition descriptors
            sb_q = persist.tile([P, NT, D], F16)
            sb_k = persist.tile([P, NT, D], F16)
            nc.sync.dma_start(out=sb_k, in_=k_r[:, :, :])
            nc.sync.dma_start(out=sb_q, in_=q_r[:, :, :])
            # v: fp32 exact
            sb_v = persist.tile([P, NVT, D], F32)
            nc.sync.dma_start(out=sb_v, in_=v_r[:, :, :])

            # HAM warm-up: ~3.4us of dummy PE work during the DMA wait so
            # the real matmuls run at 2.4GHz instead of the cold 1.2GHz
            wsrc = persist.tile([P, 2 * D], F16)
            nc.vector.memset(wsrc, 0.0)
            for w in range(16):
                ps_w = ps_mid.tile([P, 2 * D], F32, tag="mid", name=f"ps_w{w}")
                nc.tensor.matmul(ps_w, lhsT=wsrc[:, 0:D], rhs=wsrc,
                                 start=True, stop=True)

            # ---- phase 1 (PE): k^T k first, then q^T k / q^T q ----
            # one PSUM bank per accumulation group: a start=True clear is
            # bank-granular and wipes a sibling group's has_written bits.
            # kk finishes first so the whole rnk sub-chain (diag, sqrt,
            # reciprocal on DVE/ACT) overlaps the remaining matmuls.
            ps_qk = ps_acc.tile([P, D], F32)
            ps_qq = ps_acc.tile([P, D], F32)
            ps_kk = ps_acc.tile([P, D], F32)
            for t in range(NT):
                kt = sb_k[:, t, :]
                nc.tensor.matmul(ps_kk, lhsT=kt, rhs=kt,
                                 start=(t == 0), stop=(t == NT - 1))
            for t in range(NT):
                qt = sb_q[:, t, :]
                kt = sb_k[:, t, :]
                st, sp = (t == 0), (t == NT - 1)
                nc.tensor.matmul(ps_qk, lhsT=qt, rhs=kt, start=st, stop=sp)
                nc.tensor.matmul(ps_qq, lhsT=qt, rhs=qt, start=st, stop=sp)

            # warm the ACT Sqrt/Exp tables off the critical path (cold-table
            # ACTIVATE costs ~1-2us)
            # (input tied to sb_k so the scheduler cannot hoist these ACT
            # ops ahead of the k DMA issue on the same engine)
            warm2 = work.tile([P, 1], F32, name="warm2")
            nc.scalar.activation(out=warm2, in_=sb_k[:, 0, 0:1],
                                 func=mybir.ActivationFunctionType.Sqrt)
            warm3 = work.tile([P, 1], F32, name="warm3")
            nc.scalar.activation(out=warm3, in_=sb_k[:, 0, 0:1],
                                 func=mybir.ActivationFunctionType.Exp)

            # rnk sub-chain (DVE/ACT; overlaps the qk/qq matmuls above)
            dk = work.tile([P, P], F32)
            nc.vector.tensor_mul(dk, ps_kk, ident)
            sq_k = work.tile([P, 1], F32)
            nc.vector.reduce_sum(sq_k, dk, axis=mybir.AxisListType.X)
            nk = work.tile([P, 1], F32)
            nc.scalar.activation(out=nk, in_=sq_k,
                                 func=mybir.ActivationFunctionType.Sqrt)
            rnk = work.tile([P, 1], F32)
            nc.vector.reciprocal(rnk, nk)

            # rnq sub-chain
            dq = work.tile([P, P], F32)
            nc.vector.tensor_mul(dq, ps_qq, ident)
            sq_q = work.tile([P, 1], F32)
            nc.vector.reduce_sum(sq_q, dq, axis=mybir.AxisListType.X)
            nq = work.tile([P, 1], F32)
            nc.scalar.activation(out=nq, in_=sq_q,
                                 func=mybir.ActivationFunctionType.Sqrt)
            rnq = work.tile([P, 1], F32)
            nc.vector.reciprocal(rnq, nq)

            # rnk as a broadcast matrix: transpose to a row, outer with ones
            ps_rT = ps_mid.tile([1, P], F32, tag="mid", name="ps_rT")
            nc.tensor.transpose(ps_rT, rnk, ident)
            rnk_row = work.tile([1, P], F32)
            nc.vector.tensor_copy(rnk_row, ps_rT)
            ps_bc = ps_mid.tile([P, P], F32, tag="mid", name="ps_bc")
            nc.tensor.matmul(ps_bc, lhsT=ones_row, rhs=rnk_row,
                             start=True, stop=True)
            rnk_b = work.tile([P, P], F32)
            nc.vector.tensor_copy(rnk_b, ps_bc)

            # ---- v transposes (PE, fp32 exact): vT[:, s, :] = v_s^T ----
            sb_vT = persist.tile([P, NVT, D], F32)
            for s in range(NVT):
                ps_vT = ps_mm.tile([P, P], F32, tag="vT")
                nc.tensor.transpose(ps_vT, sb_v[:, s, :], ident)
                nc.vector.tensor_copy(sb_vT[:, s, :], ps_vT)

            # ---- softmax over e (free axis) ----
            qks = work.tile([P, P], F32)
            nc.vector.tensor_mul(qks, ps_qk, rnk_b)
            E = work.tile([P, P], F32)
            S = work.tile([P, 1], F32)
            nc.scalar.activation(out=E, in_=qks,
                                 func=mybir.ActivationFunctionType.Exp,
                                 scale=rnq, accum_out=S)
            rS = work.tile([P, 1], F32)
            nc.vector.reciprocal(rS, S)
            sm = work.tile([P, P], F32)
            nc.vector.tensor_scalar_mul(sm, E, rS)
            ps_smT = ps_mid.tile([P, P], F32, tag="mid", name="ps_smT")
            nc.tensor.transpose(ps_smT, sm, ident)
            smT = persist.tile([P, P], F32)   # [e, d]
            nc.vector.tensor_copy(smT, ps_smT)

            # ---- phase 2 (PE, fp32): out_s[l, d] = vT_s^T @ smT ----
            sb_out = persist.tile([P, NVT, D], F32)
            for s in range(NVT):
                ps_o = ps_mm.tile([P, P], F32, tag="po")
                nc.tensor.matmul(ps_o, lhsT=sb_vT[:, s, :], rhs=smT,
                                 start=True, stop=True)
                nc.vector.tensor_copy(sb_out[:, s, :], ps_o)
                if s == NVT // 2 - 1:
                    nc.scalar.dma_start(out=o_r[:, 0:NVT // 2, :],
                                        in_=sb_out[:, 0:NVT // 2, :])
                elif s == NVT - 1:
                    nc.scalar.dma_start(out=o_r[:, NVT // 2:, :],
                                        in_=sb_out[:, NVT // 2:, :])
    nc.compile()
    return nc


_CACHE: dict = {}


def _get_nc() -> bass.Bass:
    if "nc" not in _CACHE:
        _CACHE["nc"] = _build()
    return _CACHE["nc"]


def kernel(q: np.ndarray, k: np.ndarray, v: np.ndarray) -> np.ndarray:
    nc = _get_nc()
    q = np.ascontiguousarray(np.asarray(q, dtype=np.float32).astype(np.float16))
    k = np.ascontiguousarray(np.asarray(k, dtype=np.float32).astype(np.float16))
    v = np.ascontiguousarray(np.asarray(v, dtype=np.float32))
    in_maps = []
    for c in range(NCORES):
        b, h = divmod(c, 2)
        in_maps.append({
            "q": q[b],
            "k": k[b],
            "v": np.ascontiguousarray(v[b, h * LV:(h + 1) * LV]),
        })
    res = run_bass_kernel_spmd(nc, in_maps, list(range(NCORES))).results
    out = np.empty((B, L, D), dtype=np.float32)
    for c in range(NCORES):
        b, h = divmod(c, 2)
        out[b, h * LV:(h + 1) * LV] = res[c]["out"]
    return out
